# revision 31
# baseline (speedup 1.0000x reference)
"""Trainium2 Bass kernel for nn_CompNet (spiking LIF RNN).

Math summary (reformulation of the reference):
  Per step t:  h = W1 x_t + b1;  i = Wr [h; y] + br
               v1 <- 0.5 v1 + 0.5 i ; s1 = (v1>=1); v1 *= (1-s1)
               logits = W2 s1 + b2
               v2 <- 0.5 v2 + 0.5 logits ; s2 = (v2>=1); v2 *= (1-s2)
  out = mean_{t>=15} s2                                    -> (B, C)

Key algebraic folds (all host-side, exact in fp32):
  * h only enters via Wr_h @ h, so fold:  Wtil = -0.5*Wr_h@W1  (64x700)
  * substitute s = 1 - m with m = (v < 1), folding the constant
    Wr_y@1 / W2@1 terms into per-population biases; the bias itself is
    folded into the big matmul via a ones-row of x (row 700 of the
    padded 768-dim input is 1.0, the matching Wt row holds 1 - bt).
  * LIF1 (rows 0..63) and LIF2 (rows 64..83) are stacked into one 84-row
    population, with LIF2 lagging one step.

Pipeline per 512-column slice (16 steps) of the drive matmul:
  x DMA -> PE (6 k-chunk matmuls -> PSUM) -> ACT copy PSUM->SBUF (E, bf16)
spread evenly between loop steps so PE/ACT work hides in the serial
chain's latency.  The sequential loop (per step j):
  psum_j = I84@E_j + L@M_j  (PE)
  M_{j+1} = (0.5*cu < psum)  (DVE stt, PE produced qsum = 1 - v)
  v = 0.5*cu - qsum ; cu' = (v+1)*M_{j+1}   (DVE stt x2, off-chain)
Output: blocks 17..251 of M rows 64..83 are summed by in-place log-
halving bf16 adds (integer-exact up to 255), then out = (S-235)*(-1/235).

Sharding: pure data parallelism, batch 256 -> 8 cores x 32.
"""

import numpy as np
import ml_dtypes

BF16 = ml_dtypes.bfloat16

B, T, D, H, C = 256, 250, 700, 64, 20
NCORES = 8
BL = B // NCORES          # 32 batch per core
P = H + C                 # 84 stacked feature rows
KCH = 6                   # ceil(700/128) contraction chunks
DP = KCH * 128            # 768 padded feature dim (700 data + ones row)
NCOL = T * BL             # 8000 drive columns per core
NCOL2 = NCOL + BL         # + bias-only block feeding loop iter 250
SLICE = 512               # drive matmul slice (16 steps)
VTH_INIT = 2.0e9          # suppresses the phantom LIF2 step at j=0

_CACHE = {}


def _build_nc():
    import concourse.bass as bass
    import concourse.mybir as mybir
    from concourse.tile import TileContext

    dt = mybir.dt
    OP = mybir.AluOpType
    ts = bass.ts

    # detect_race_conditions=False: we strip same-engine self-waits
    # (walrus one-wait-per-instruction limit); each engine executes its
    # stream in order on silicon, so those windows cannot race.
    nc = bass.Bass(
        "TRN2", target_bir_lowering=False, debug=False,
        detect_race_conditions=False,
    )

    xT = nc.dram_tensor("xT", [KCH, 128, NCOL2], dt.bfloat16, kind="ExternalInput").ap()
    Wt = nc.dram_tensor("Wt", [KCH, 128, P], dt.bfloat16, kind="ExternalInput").ap()
    Lw = nc.dram_tensor("Lw", [H, P], dt.bfloat16, kind="ExternalInput").ap()
    I84 = nc.dram_tensor("I84", [P, P], dt.bfloat16, kind="ExternalInput").ap()
    out_d = nc.dram_tensor("out", [C, BL], dt.float32, kind="ExternalOutput").ap()

    # column slices of the big matmul / x streaming
    slices = []
    c0 = 0
    while c0 < NCOL2:
        w = min(SLICE, NCOL2 - c0)
        slices.append((c0, w))
        c0 += w
    NSLICE = len(slices)

    with TileContext(nc) as tc:
        with (
            tc.tile_pool(name="const", bufs=1) as cp,
            tc.tile_pool(name="xs", bufs=8) as xp,
            tc.tile_pool(name="wk", bufs=4) as wp,
            tc.tile_pool(name="psA", bufs=2, space="PSUM") as psA,
            tc.tile_pool(name="psL", bufs=3, space="PSUM") as psL,
        ):
            # ---- persistent tiles ----
            E_t = cp.tile([P, NCOL2], dt.bfloat16, tag="E")            # blocks 0..250
            M_t = cp.tile([P, NCOL2 + BL], dt.bfloat16, tag="M")       # blocks 0..251
            wts_all = cp.tile([128, KCH * P], dt.bfloat16, tag="wall")
            L_t = cp.tile([H, P], dt.bfloat16, tag="L")
            I_t = cp.tile([P, P], dt.bfloat16, tag="I")
            cu0 = cp.tile([P, BL], dt.float32, tag="cu0")
            R_t = cp.tile([128, BL], dt.float32, tag="R")

            # ---- prologue: inits ----
            wts = [wts_all[:, k * P:(k + 1) * P] for k in range(KCH)]

            def emit_wdma():
                # Single DMA for all 6 weight chunks: each dma_start costs
                # ~650ns of serialized issue time on the sync queue.
                nc.sync.dma_start(
                    out=wts_all[:, :].rearrange("p (k c) -> p k c", k=KCH),
                    in_=Wt[:, :, :].rearrange("k p c -> p k c"),
                )
                nc.sync.dma_start(out=L_t[:, :], in_=Lw[:, :])
                nc.sync.dma_start(out=I_t[:, :], in_=I84[:, :])

            nc.vector.memset(M_t[0:H, 0:BL], 1.0)     # m_{-1} = 1 (y=0)
            nc.vector.memset(M_t[H:P, 0:BL], 0.0)
            nc.vector.memset(cu0[0:H, :], 0.0)        # v1 carry starts at 0
            nc.vector.memset(cu0[H:P, :], VTH_INIT)   # kill phantom LIF2 step

            # ---- x DMAs (paired slices, ONE issue each) + big matmul ----
            PAIR = 2 * SLICE
            xtiles = {}

            def emit_xdma(p):
                c0 = p * PAIR
                w = min(PAIR, NCOL2 - c0)
                t = xp.tile([128, KCH * PAIR], dt.bfloat16, tag="xp")
                nc.sync.dma_start(
                    out=t[:, :].rearrange(
                        "p (k c) -> p k c", k=KCH
                    )[:, :, 0:w],
                    in_=xT[:, :, c0:c0 + w].rearrange("k p c -> p k c"),
                )
                xtiles[p] = t

            def pair_ops(q):
                """Yield thunks: 12 matmuls (2 slices x 6 k-chunks) + ONE
                copy for x pair q.  The pa tile spans TWO PSUM banks so a
                single DVE copy evacuates 1024 columns ((120+1024)/0.96 =
                1192ns vs 2x687ns), and the copy count (DVE-FIFO
                insertions in the front-loaded burst) halves."""
                c0 = q * PAIR
                wq = min(PAIR, NCOL2 - c0)
                xt = xtiles  # late-bound: tile looked up at emission time
                pa = psA.tile([P, PAIR], dt.float32, tag="pa")

                def mk_mm(h, k, pa=pa):
                    w = min(SLICE, wq - h * SLICE)

                    def f():
                        nc.tensor.matmul(
                            out=pa[:, h * SLICE:h * SLICE + w],
                            lhsT=wts[k],
                            rhs=xt[q][:, h * SLICE + k * PAIR:
                                      h * SLICE + k * PAIR + w],
                            start=(k == 0), stop=(k == KCH - 1),
                        )
                    return f

                for h in range(2):
                    if h * SLICE >= wq:
                        continue
                    for k in range(KCH):
                        yield mk_mm(h, k)

                def mk_copy(pa=pa, c0=c0, wq=wq):
                    # drive (incl. folded bias) PSUM -> SBUF bf16.  On DVE
                    # (not ACT): the psA-bank WAR then rides the DVE sem,
                    # which Tile subsumes into the loop's existing mask
                    # waits, keeping every PE matmul at ONE sync wait
                    # (walrus limit).
                    def f():
                        nc.vector.tensor_scalar(
                            out=E_t[:, c0:c0 + wq], in0=pa[:, 0:wq],
                            scalar1=1.0, scalar2=None, op0=OP.mult,
                        )
                    return f

                yield mk_copy()

            # prologue: weights first (tiny, they gate the warmup and the
            # first drive matmuls), then the two x pairs the loop start
            # needs, then PE warmup while x streams, then slice 0.
            emit_wdma()
            emit_xdma(0)
            emit_xdma(1)
            # Back-to-back dummy matmuls (into the psA pool's banks, no
            # extra PSUM) while the first x pair streams in: pushes HAM
            # past its 3.4us busy window so the front-loaded drive
            # matmul burst runs warm.
            for i in range(16):
                pw = psA.tile([P, PAIR], dt.float32, tag="pa")
                nc.tensor.matmul(
                    out=pw[:, 0:KCH * P], lhsT=wts[0],
                    rhs=wts_all[:, :], start=True, stop=True,
                )
            for th in pair_ops(0):
                th()

            # remaining x pairs: issue them all now — the sync queue
            # serializes dma_start issues (~650ns each), so doing it here
            # keeps those stalls out of the loop; transfers complete long
            # before their slices need them.  Tile's scheduler front-loads
            # the drive-slice computation into the first ~15 steps anyway
            # (throttled by the 2-bank psA rotation), which is near-optimal
            # given the E-copies saturate the DVE regardless of placement.
            npair = (NCOL2 + PAIR - 1) // PAIR
            for p in range(2, npair):
                emit_xdma(p)
            extras = {}
            for q in range(1, npair):
                base = 32 * (q - 1) + 1
                for i, th in enumerate(pair_ops(q)):
                    extras.setdefault(base + 2 * i, []).append(th)

            # ---- the sequential LIF loop ----
            cu_prev = cu0
            for j in range(T + 1):
                for th in extras.pop(j, []):
                    th()
                ps = psL.tile([P, BL], dt.float32, tag="ps")
                nc.tensor.matmul(
                    out=ps[:, :], lhsT=I_t[:, :], rhs=E_t[:, ts(j, BL)],
                    start=True, stop=False,
                )
                nc.tensor.matmul(
                    out=ps[:, :], lhsT=L_t[:, :], rhs=M_t[0:H, ts(j, BL)],
                    start=False, stop=True,
                )
                # PE produced qsum = 1 - (drive + recurrent) so the spike
                # mask comes straight off PSUM in ONE fused op:
                #   v < 1  <=>  0.5*cu < qsum.
                nc.vector.scalar_tensor_tensor(
                    out=M_t[:, ts(j + 1, BL)], in0=cu_prev[:, :], scalar=0.5,
                    in1=ps[:, :], op0=OP.mult, op1=OP.is_lt,
                )
                if j < T:
                    v = wp.tile([P, BL], dt.float32, tag="v")
                    # u = v - 1 = 0.5*cu - qsum
                    nc.vector.scalar_tensor_tensor(
                        out=v[:, :], in0=cu_prev[:, :], scalar=0.5,
                        in1=ps[:, :], op0=OP.mult, op1=OP.subtract,
                    )
                    cu = wp.tile([P, BL], dt.float32, tag="cu")
                    # cu = v*m = (u + 1)*m
                    nc.vector.scalar_tensor_tensor(
                        out=cu[:, :], in0=v[:, :], scalar=1.0,
                        in1=M_t[:, ts(j + 1, BL)], op0=OP.add, op1=OP.mult,
                    )
                    cu_prev = cu
            for jj in sorted(extras):
                for th in extras[jj]:
                    th()

            # ---- tail: S = sum_t m2 over blocks 17..251 by log-halving
            # adds (bf16 integer-exact to 255) ping-ponged through two
            # scratch buffers (never in-place: DVE streaming in-place
            # read-write is a silicon hazard), then out = (S-235)*(-1/235).
            sc0 = cp.tile([C, 118 * BL], dt.bfloat16, tag="sc0", name="sc0")
            sc1 = cp.tile([C, 60 * BL], dt.bfloat16, tag="sc1", name="sc1")
            sc = [sc0, sc1]

            def fold(src_ap, n, dst):
                # dst[0:lo) = src[0:h) + src[lo:n), middle block copied
                h = n // 2
                lo = n - h
                nc.vector.tensor_tensor(
                    out=dst[:, 0:h * BL], in0=src_ap(0, h),
                    in1=src_ap(lo, n), op=OP.add,
                )
                if lo != h:
                    nc.vector.tensor_scalar(
                        out=dst[:, h * BL:lo * BL], in0=src_ap(h, lo),
                        scalar1=1.0, scalar2=None, op0=OP.mult,
                    )
                return lo

            off = 17
            n = fold(
                lambda a, b: M_t[H:P, (off + a) * BL:(off + b) * BL],
                235, sc[0],
            )
            cur = 0
            while n > 1:
                n = fold(
                    lambda a, b, cur=cur: sc[cur][:, a * BL:b * BL],
                    n, sc[1 - cur],
                )
                cur = 1 - cur
            nc.vector.tensor_scalar(
                out=R_t[H:P, :], in0=sc[cur][:, 0:BL],
                scalar1=235.0, scalar2=-1.0 / 235.0,
                op0=OP.subtract, op1=OP.mult,
            )
            nc.sync.dma_start(out=out_d[:, :], in_=R_t[H:P, 0:BL])

    _strip_self_waits(nc)
    return nc


def _strip_self_waits(nc):
    """walrus in this container accepts only ONE sync wait per compute
    instruction (AC/MM/STT structs), and same-engine sem waits cost real
    propagation latency (~100-240ns observed) even though engine streams
    execute in order.  Drop every wait an instruction holds on a
    semaphore lane that it also updates itself (self-wait): engine
    in-order execution already guarantees those.  For instructions still
    holding >1 wait, drop the same-engine ones.  SP/DMA/Drain
    instructions support multi-wait and are handled as before."""
    import concourse.mybir as mybir

    out_names = set()
    for alloc in nc.m.functions[0].allocations:
        if (
            isinstance(alloc, mybir.MemoryLocationSet)
            and alloc.kind == "ExternalOutput"
        ):
            for ml in alloc.memorylocations:
                out_names.add(ml.name)
    keep_lanes = set()
    for name, inst in nc.inst_map.items():
        if "DMA" not in type(inst).__name__:
            continue
        c = inst.concise()
        if any(f"@{n}" in c.split("in=")[0] for n in out_names):
            for u in (inst.sync_info.on_update or []) if inst.sync_info else []:
                keep_lanes.add(u.ant_name)

    # Strip only instructions holding >1 wait (walrus one-wait limit),
    # dropping the same-engine (self) waits.  Single self-waits are LEFT
    # IN PLACE: stripping them races on silicon — on PE they hold the
    # LDWEIGHTS reorder window at bay, and on DVE they enforce write->
    # read visibility between nearby ops (removing them gave varying
    # wrong results on HW even though CoreSim passes).
    for name, inst in nc.inst_map.items():
        si = inst.sync_info
        if si is None or not si.on_wait or len(si.on_wait) < 2:
            continue
        own = {u.ant_name for u in (si.on_update or [])}
        kept = [w for w in si.on_wait if w.ant_name not in own]
        if "Drain" in type(inst).__name__ and len(kept) > 1:
            # Tail drain: engine completion is already enforced by the
            # all-engine barrier that follows; only output-DMA lanes
            # need the drain.
            kept = [w for w in kept if w.ant_name in keep_lanes]
        if len(kept) != len(si.on_wait):
            si.on_wait = kept


def _prep_shared(W1, b1, Wr, br, W2, b2):
    f32 = np.float32
    W1 = np.asarray(W1, f32); b1 = np.asarray(b1, f32)
    Wr = np.asarray(Wr, f32); br = np.asarray(br, f32)
    W2 = np.asarray(W2, f32); b2 = np.asarray(b2, f32)
    Wrh, Wry = Wr[:, :H], Wr[:, H:]
    # Negated ("qsum = 1 - v") encoding: PE computes q = (1-bt) - Wtil@x
    # - 0.5*[Wry;W2]@m with m in {0,1}; spike test is then 0.5*cu < q.
    Wtil = -0.5 * (Wrh @ W1)                                  # [64, 700]
    bt1 = 0.5 * (Wrh @ b1 + br + Wry.sum(axis=1))
    bt2 = 0.5 * (b2 + W2.sum(axis=1))
    bfl = 1.0 - np.concatenate([bt1, bt2])                    # [84]
    Wtp = np.zeros((P, DP), f32)
    Wtp[:H, :D] = Wtil
    Wtp[:, D] = bfl          # bias rides the ones-row of x (row 700)
    Wt6 = np.ascontiguousarray(
        Wtp.reshape(P, KCH, 128).transpose(1, 2, 0)
    ).astype(BF16)                                            # [6, 128, 84]
    L = np.concatenate([0.5 * Wry.T, 0.5 * W2.T], axis=1).astype(BF16)
    I84 = np.eye(P, dtype=f32).astype(BF16)
    return Wt6, L, I84


def _ensure_ntff_hook():
    """The RL container's antenv stub lacks axon_hooks; bass_utils imports it
    unconditionally when tracing. Register the ctypes-based hook ourselves."""
    import sys
    import types
    try:
        import antenv
        if "antenv.axon_hooks" in sys.modules:
            return
        mod = types.ModuleType("antenv.axon_hooks")
        _h = [None]
        mod.set_axon_ntff_profile_hook = lambda h: _h.__setitem__(0, h)
        mod.get_axon_ntff_profile_hook = lambda: _h[0]
        sys.modules["antenv.axon_hooks"] = mod
        antenv.axon_hooks = mod
        try:
            from trn_agent_boot.trn_boot import _ntff_profile_via_ctypes
            mod.set_axon_ntff_profile_hook(
                _ntff_profile_via_ctypes("/opt/axon/libaxon_pjrt.so")
            )
        except Exception:
            pass
    except Exception:
        pass


def kernel(x, W1, b1, Wr, br, W2, b2):
    from concourse.bass_utils import run_bass_kernel_spmd

    _ensure_ntff_hook()

    if "nc" not in _CACHE:
        _CACHE["nc"] = _build_nc()
    nc = _CACHE["nc"]

    Wt6, L, I84 = _prep_shared(W1, b1, Wr, br, W2, b2)

    x = np.asarray(x, np.float32)
    xbf = x.astype(BF16)                                      # (B, T, D)
    in_maps = []
    for c in range(NCORES):
        xc = xbf[c * BL:(c + 1) * BL]                         # (32, 250, 700)
        xt = np.zeros((DP, NCOL2), BF16)
        xt[:D, :NCOL] = xc.transpose(2, 1, 0).reshape(D, NCOL)
        xt[D, :] = BF16(1.0)       # ones-row: bias for every column
        in_maps.append({
            "xT": np.ascontiguousarray(xt.reshape(KCH, 128, NCOL2)),
            "Wt": Wt6, "Lw": L, "I84": I84,
        })

    res = run_bass_kernel_spmd(nc, in_maps, core_ids=list(range(NCORES)))
    _CACHE["last_results"] = res
    out = np.concatenate(
        [np.asarray(r["out"]).T for r in res.results], axis=0
    ).astype(np.float32)                                      # (256, 20)
    return out


# revision 32
# speedup vs baseline: 1.0428x; 1.0428x over previous
"""Trainium2 Bass kernel for nn_CompNet (spiking LIF RNN).

Math summary (reformulation of the reference):
  Per step t:  h = W1 x_t + b1;  i = Wr [h; y] + br
               v1 <- 0.5 v1 + 0.5 i ; s1 = (v1>=1); v1 *= (1-s1)
               logits = W2 s1 + b2
               v2 <- 0.5 v2 + 0.5 logits ; s2 = (v2>=1); v2 *= (1-s2)
  out = mean_{t>=15} s2                                    -> (B, C)

Key algebraic folds (all host-side, exact in fp32):
  * h only enters via Wr_h @ h, so fold:  Wtil = -0.5*Wr_h@W1  (64x700)
  * substitute s = 1 - m with m = (v < 1), folding the constant
    Wr_y@1 / W2@1 terms into per-population biases; the bias itself is
    folded into the big matmul via a ones-row of x (row 700 of the
    padded 768-dim input is 1.0, the matching Wt row holds 1 - bt).
  * LIF1 (rows 0..63) and LIF2 (rows 64..83) are stacked into one 84-row
    population, with LIF2 lagging one step.

Pipeline per 512-column slice (16 steps) of the drive matmul:
  x DMA -> PE (6 k-chunk matmuls -> PSUM) -> ACT copy PSUM->SBUF (E, bf16)
spread evenly between loop steps so PE/ACT work hides in the serial
chain's latency.  The sequential loop (per step j):
  psum_j = I84@E_j + L@M_j  (PE)
  M_{j+1} = (0.5*cu < psum)  (DVE stt, PE produced qsum = 1 - v)
  v = 0.5*cu - qsum ; cu' = (v+1)*M_{j+1}   (DVE stt x2, off-chain)
Output: blocks 17..251 of M rows 64..83 are summed by in-place log-
halving bf16 adds (integer-exact up to 255), then out = (S-235)*(-1/235).

Sharding: pure data parallelism, batch 256 -> 8 cores x 32.
"""

import numpy as np
import ml_dtypes

BF16 = ml_dtypes.bfloat16

B, T, D, H, C = 256, 250, 700, 64, 20
NCORES = 8
BL = B // NCORES          # 32 batch per core
P = H + C                 # 84 stacked feature rows
KCH = 6                   # ceil(700/128) contraction chunks
DP = KCH * 128            # 768 padded feature dim (700 data + ones row)
NCOL = T * BL             # 8000 drive columns per core
NCOL2 = NCOL + BL         # + bias-only block feeding loop iter 250
SLICE = 512               # drive matmul slice (16 steps)
VTH_INIT = 2.0e9          # suppresses the phantom LIF2 step at j=0

_CACHE = {}


def _build_nc():
    import concourse.bass as bass
    import concourse.mybir as mybir
    from concourse.tile import TileContext

    dt = mybir.dt
    OP = mybir.AluOpType
    ts = bass.ts

    # detect_race_conditions=False: we strip same-engine self-waits
    # (walrus one-wait-per-instruction limit); each engine executes its
    # stream in order on silicon, so those windows cannot race.
    nc = bass.Bass(
        "TRN2", target_bir_lowering=False, debug=False,
        detect_race_conditions=False,
    )

    xT = nc.dram_tensor("xT", [KCH, 128, NCOL2], dt.bfloat16, kind="ExternalInput").ap()
    Wt = nc.dram_tensor("Wt", [KCH, 128, P], dt.bfloat16, kind="ExternalInput").ap()
    Lw = nc.dram_tensor("Lw", [H, P], dt.bfloat16, kind="ExternalInput").ap()
    I84 = nc.dram_tensor("I84", [P, P], dt.bfloat16, kind="ExternalInput").ap()
    out_d = nc.dram_tensor("out", [C, BL], dt.float32, kind="ExternalOutput").ap()

    # column slices of the big matmul / x streaming
    slices = []
    c0 = 0
    while c0 < NCOL2:
        w = min(SLICE, NCOL2 - c0)
        slices.append((c0, w))
        c0 += w
    NSLICE = len(slices)

    with TileContext(nc) as tc:
        with (
            tc.tile_pool(name="const", bufs=1) as cp,
            tc.tile_pool(name="xs", bufs=8) as xp,
            tc.tile_pool(name="wk", bufs=4) as wp,
            tc.tile_pool(name="psA", bufs=2, space="PSUM") as psA,
            tc.tile_pool(name="psL", bufs=4, space="PSUM") as psL,
        ):
            # ---- persistent tiles ----
            E_t = cp.tile([P, NCOL2], dt.bfloat16, tag="E")            # blocks 0..250
            M_t = cp.tile([P, NCOL2 + BL], dt.bfloat16, tag="M")       # blocks 0..251
            wts_all = cp.tile([128, KCH * P], dt.bfloat16, tag="wall")
            L_t = cp.tile([H, P], dt.bfloat16, tag="L")
            I_t = cp.tile([P, P], dt.bfloat16, tag="I")
            cu0 = cp.tile([P, BL], dt.float32, tag="cu0")
            R_t = cp.tile([128, BL], dt.float32, tag="R")

            # ---- prologue: inits ----
            wts = [wts_all[:, k * P:(k + 1) * P] for k in range(KCH)]

            def emit_wdma():
                # Single DMA for all 6 weight chunks: each dma_start costs
                # ~650ns of serialized issue time on the sync queue.
                nc.sync.dma_start(
                    out=wts_all[:, :].rearrange("p (k c) -> p k c", k=KCH),
                    in_=Wt[:, :, :].rearrange("k p c -> p k c"),
                )
                nc.sync.dma_start(out=L_t[:, :], in_=Lw[:, :])
                nc.sync.dma_start(out=I_t[:, :], in_=I84[:, :])

            nc.vector.memset(M_t[0:H, 0:BL], 1.0)     # m_{-1} = 1 (y=0)
            nc.vector.memset(M_t[H:P, 0:BL], 0.0)
            nc.vector.memset(cu0[0:H, :], 0.0)        # v1 carry starts at 0
            nc.vector.memset(cu0[H:P, :], VTH_INIT)   # kill phantom LIF2 step

            # ---- x DMAs (paired slices, ONE issue each) + big matmul ----
            PAIR = 2 * SLICE
            xtiles = {}

            def emit_xdma(p):
                c0 = p * PAIR
                w = min(PAIR, NCOL2 - c0)
                t = xp.tile([128, KCH * PAIR], dt.bfloat16, tag="xp")
                nc.sync.dma_start(
                    out=t[:, :].rearrange(
                        "p (k c) -> p k c", k=KCH
                    )[:, :, 0:w],
                    in_=xT[:, :, c0:c0 + w].rearrange("k p c -> p k c"),
                )
                xtiles[p] = t

            def slice_ops(s):
                """Yield thunks: 6 matmuls + 1 copy for slice s."""
                c0, w = slices[s]
                xt = xtiles  # late-bound: tile looked up at emission time
                pa = psA.tile([P, SLICE], dt.float32, tag="pa")

                def mk_mm(k, pa=pa, w=w, s=s):
                    def f():
                        off = (s % 2) * SLICE + k * PAIR
                        nc.tensor.matmul(
                            out=pa[:, 0:w], lhsT=wts[k],
                            rhs=xt[s // 2][:, off:off + w],
                            start=(k == 0), stop=(k == KCH - 1),
                        )
                    return f

                for k in range(KCH):
                    yield mk_mm(k)

                def mk_copy(pa=pa, c0=c0, w=w):
                    # drive (incl. folded bias) PSUM -> SBUF bf16.  On DVE
                    # (not ACT): the psA-bank WAR then rides the DVE sem,
                    # which Tile subsumes into the loop's existing mask
                    # waits, keeping every PE matmul at ONE sync wait
                    # (walrus limit).  Costs ~390ns of DVE time per 16
                    # steps, absorbed by the chain's slack.
                    def f():
                        nc.vector.tensor_scalar(
                            out=E_t[:, c0:c0 + w], in0=pa[:, 0:w],
                            scalar1=1.0, scalar2=None, op0=OP.mult,
                        )
                    return f

                yield mk_copy()

            # prologue: weights first (tiny, they gate the warmup and the
            # first drive matmuls), then the two x pairs the loop start
            # needs, then PE warmup while x streams, then slice 0.
            emit_wdma()
            emit_xdma(0)
            emit_xdma(1)
            # Back-to-back dummy matmuls (into the psA pool's banks, no
            # extra PSUM) while the first x pair streams in: pushes HAM
            # past its 3.4us busy window so the front-loaded drive
            # matmul burst runs warm.
            for i in range(16):
                pw = psA.tile([P, SLICE], dt.float32, tag="pa")
                nc.tensor.matmul(
                    out=pw[:, 0:KCH * P], lhsT=wts[0],
                    rhs=wts_all[:, :], start=True, stop=True,
                )
            for th in slice_ops(0):
                th()

            # remaining x pairs: issue them all now — the sync queue
            # serializes dma_start issues (~650ns each), so doing it here
            # keeps those stalls out of the loop; transfers complete long
            # before their slices need them.  Tile's scheduler front-loads
            # the drive-slice computation into the first ~15 steps anyway
            # (throttled by the 2-bank psA rotation), which is near-optimal
            # given the E-copies saturate the DVE regardless of placement.
            npair = (NCOL2 + PAIR - 1) // PAIR
            for p in range(2, npair):
                emit_xdma(p)
            extras = {}
            for s in range(1, NSLICE):
                base = 16 * (s - 1) + 1
                for i, th in enumerate(slice_ops(s)):
                    extras.setdefault(base + 2 * i, []).append(th)

            # ---- the sequential LIF loop ----
            cu_prev = cu0
            for j in range(T + 1):
                for th in extras.pop(j, []):
                    th()
                ps = psL.tile([P, BL], dt.float32, tag="ps")
                nc.tensor.matmul(
                    out=ps[:, :], lhsT=I_t[:, :], rhs=E_t[:, ts(j, BL)],
                    start=True, stop=False,
                )
                nc.tensor.matmul(
                    out=ps[:, :], lhsT=L_t[:, :], rhs=M_t[0:H, ts(j, BL)],
                    start=False, stop=True,
                )
                # PE produced qsum = 1 - (drive + recurrent) so the spike
                # mask comes straight off PSUM in ONE fused op:
                #   v < 1  <=>  0.5*cu < qsum.
                nc.vector.scalar_tensor_tensor(
                    out=M_t[:, ts(j + 1, BL)], in0=cu_prev[:, :], scalar=0.5,
                    in1=ps[:, :], op0=OP.mult, op1=OP.is_lt,
                )
                if j < T:
                    v = wp.tile([P, BL], dt.float32, tag="v")
                    # u = v - 1 = 0.5*cu - qsum
                    nc.vector.scalar_tensor_tensor(
                        out=v[:, :], in0=cu_prev[:, :], scalar=0.5,
                        in1=ps[:, :], op0=OP.mult, op1=OP.subtract,
                    )
                    cu = wp.tile([P, BL], dt.float32, tag="cu")
                    # cu = v*m = (u + 1)*m
                    nc.vector.scalar_tensor_tensor(
                        out=cu[:, :], in0=v[:, :], scalar=1.0,
                        in1=M_t[:, ts(j + 1, BL)], op0=OP.add, op1=OP.mult,
                    )
                    cu_prev = cu
            for jj in sorted(extras):
                for th in extras[jj]:
                    th()

            # ---- tail: S = sum_t m2 over blocks 17..251 by log-halving
            # adds (bf16 integer-exact to 255) ping-ponged through two
            # scratch buffers (never in-place: DVE streaming in-place
            # read-write is a silicon hazard), then out = (S-235)*(-1/235).
            sc0 = cp.tile([C, 118 * BL], dt.bfloat16, tag="sc0", name="sc0")
            sc1 = cp.tile([C, 60 * BL], dt.bfloat16, tag="sc1", name="sc1")
            sc = [sc0, sc1]

            def fold(src_ap, n, dst):
                # dst[0:lo) = src[0:h) + src[lo:n), middle block copied
                h = n // 2
                lo = n - h
                nc.vector.tensor_tensor(
                    out=dst[:, 0:h * BL], in0=src_ap(0, h),
                    in1=src_ap(lo, n), op=OP.add,
                )
                if lo != h:
                    nc.vector.tensor_scalar(
                        out=dst[:, h * BL:lo * BL], in0=src_ap(h, lo),
                        scalar1=1.0, scalar2=None, op0=OP.mult,
                    )
                return lo

            off = 17
            n = fold(
                lambda a, b: M_t[H:P, (off + a) * BL:(off + b) * BL],
                235, sc[0],
            )
            cur = 0
            while n > 1:
                n = fold(
                    lambda a, b, cur=cur: sc[cur][:, a * BL:b * BL],
                    n, sc[1 - cur],
                )
                cur = 1 - cur
            nc.vector.tensor_scalar(
                out=R_t[H:P, :], in0=sc[cur][:, 0:BL],
                scalar1=235.0, scalar2=-1.0 / 235.0,
                op0=OP.subtract, op1=OP.mult,
            )
            nc.sync.dma_start(out=out_d[:, :], in_=R_t[H:P, 0:BL])

    _strip_self_waits(nc)
    return nc


def _strip_self_waits(nc):
    """walrus in this container accepts only ONE sync wait per compute
    instruction (AC/MM/STT structs), and same-engine sem waits cost real
    propagation latency (~100-240ns observed) even though engine streams
    execute in order.  Drop every wait an instruction holds on a
    semaphore lane that it also updates itself (self-wait): engine
    in-order execution already guarantees those.  For instructions still
    holding >1 wait, drop the same-engine ones.  SP/DMA/Drain
    instructions support multi-wait and are handled as before."""
    import concourse.mybir as mybir

    out_names = set()
    for alloc in nc.m.functions[0].allocations:
        if (
            isinstance(alloc, mybir.MemoryLocationSet)
            and alloc.kind == "ExternalOutput"
        ):
            for ml in alloc.memorylocations:
                out_names.add(ml.name)
    keep_lanes = set()
    for name, inst in nc.inst_map.items():
        if "DMA" not in type(inst).__name__:
            continue
        c = inst.concise()
        if any(f"@{n}" in c.split("in=")[0] for n in out_names):
            for u in (inst.sync_info.on_update or []) if inst.sync_info else []:
                keep_lanes.add(u.ant_name)

    # Strip only instructions holding >1 wait (walrus one-wait limit),
    # dropping the same-engine (self) waits.  Single self-waits are LEFT
    # IN PLACE: stripping them races on silicon — on PE they hold the
    # LDWEIGHTS reorder window at bay, and on DVE they enforce write->
    # read visibility between nearby ops (removing them gave varying
    # wrong results on HW even though CoreSim passes).
    for name, inst in nc.inst_map.items():
        si = inst.sync_info
        if si is None or not si.on_wait or len(si.on_wait) < 2:
            continue
        own = {u.ant_name for u in (si.on_update or [])}
        kept = [w for w in si.on_wait if w.ant_name not in own]
        if "Drain" in type(inst).__name__ and len(kept) > 1:
            # Tail drain: engine completion is already enforced by the
            # all-engine barrier that follows; only output-DMA lanes
            # need the drain.
            kept = [w for w in kept if w.ant_name in keep_lanes]
        if len(kept) != len(si.on_wait):
            si.on_wait = kept


def _prep_shared(W1, b1, Wr, br, W2, b2):
    f32 = np.float32
    W1 = np.asarray(W1, f32); b1 = np.asarray(b1, f32)
    Wr = np.asarray(Wr, f32); br = np.asarray(br, f32)
    W2 = np.asarray(W2, f32); b2 = np.asarray(b2, f32)
    Wrh, Wry = Wr[:, :H], Wr[:, H:]
    # Negated ("qsum = 1 - v") encoding: PE computes q = (1-bt) - Wtil@x
    # - 0.5*[Wry;W2]@m with m in {0,1}; spike test is then 0.5*cu < q.
    Wtil = -0.5 * (Wrh @ W1)                                  # [64, 700]
    bt1 = 0.5 * (Wrh @ b1 + br + Wry.sum(axis=1))
    bt2 = 0.5 * (b2 + W2.sum(axis=1))
    bfl = 1.0 - np.concatenate([bt1, bt2])                    # [84]
    Wtp = np.zeros((P, DP), f32)
    Wtp[:H, :D] = Wtil
    Wtp[:, D] = bfl          # bias rides the ones-row of x (row 700)
    Wt6 = np.ascontiguousarray(
        Wtp.reshape(P, KCH, 128).transpose(1, 2, 0)
    ).astype(BF16)                                            # [6, 128, 84]
    L = np.concatenate([0.5 * Wry.T, 0.5 * W2.T], axis=1).astype(BF16)
    I84 = np.eye(P, dtype=f32).astype(BF16)
    return Wt6, L, I84


def _ensure_ntff_hook():
    """The RL container's antenv stub lacks axon_hooks; bass_utils imports it
    unconditionally when tracing. Register the ctypes-based hook ourselves."""
    import sys
    import types
    try:
        import antenv
        if "antenv.axon_hooks" in sys.modules:
            return
        mod = types.ModuleType("antenv.axon_hooks")
        _h = [None]
        mod.set_axon_ntff_profile_hook = lambda h: _h.__setitem__(0, h)
        mod.get_axon_ntff_profile_hook = lambda: _h[0]
        sys.modules["antenv.axon_hooks"] = mod
        antenv.axon_hooks = mod
        try:
            from trn_agent_boot.trn_boot import _ntff_profile_via_ctypes
            mod.set_axon_ntff_profile_hook(
                _ntff_profile_via_ctypes("/opt/axon/libaxon_pjrt.so")
            )
        except Exception:
            pass
    except Exception:
        pass


def kernel(x, W1, b1, Wr, br, W2, b2):
    from concourse.bass_utils import run_bass_kernel_spmd

    _ensure_ntff_hook()

    if "nc" not in _CACHE:
        _CACHE["nc"] = _build_nc()
    nc = _CACHE["nc"]

    Wt6, L, I84 = _prep_shared(W1, b1, Wr, br, W2, b2)

    x = np.asarray(x, np.float32)
    xbf = x.astype(BF16)                                      # (B, T, D)
    in_maps = []
    for c in range(NCORES):
        xc = xbf[c * BL:(c + 1) * BL]                         # (32, 250, 700)
        xt = np.zeros((DP, NCOL2), BF16)
        xt[:D, :NCOL] = xc.transpose(2, 1, 0).reshape(D, NCOL)
        xt[D, :] = BF16(1.0)       # ones-row: bias for every column
        in_maps.append({
            "xT": np.ascontiguousarray(xt.reshape(KCH, 128, NCOL2)),
            "Wt": Wt6, "Lw": L, "I84": I84,
        })

    res = run_bass_kernel_spmd(nc, in_maps, core_ids=list(range(NCORES)))
    _CACHE["last_results"] = res
    out = np.concatenate(
        [np.asarray(r["out"]).T for r in res.results], axis=0
    ).astype(np.float32)                                      # (256, 20)
    return out
